# revision 9
# baseline (speedup 1.0000x reference)
"""Dispersion loss kernel for 8x TRN2 NeuronCores (Bass/Tile).

Math: rows of class_centroid [8192, 2048] are L2-normalized; the loss is
  mean_i( sum_j exp(-||xn_i - xn_j||^2) / (N-1) )
    = (1/(N*(N-1))) * sum_{i,j} exp(2*cos_ij - 2)       (cos_ij = xn_i . xn_j)

Since only the total sum is needed, we exploit symmetry: with 16 row-blocks
of 512, only block pairs (b, b+d mod 16) for d=0..8 are computed (d=8 pairs
are covered twice and down-weighted on the host). Each core c loads the 10
consecutive blocks 2c..2c+9 (mod 16) and runs the SAME program (SPMD) over a
fixed list of 18 slot pairs; per-core slot->block mapping makes the union
cover all 136 unordered block pairs.

Per core: normalize+cast to bf16, DMA-transpose into feature-major layout
[128, D/128, 512] per block (SBUF-resident), then 18 * 4 PSUM tiles of 16
accumulating matmuls each; epilogue exp(2G-2) on ScalarE with fused row-sum
(min(.,1) clamp on the two diagonal tiles only). Host reduces partials in
float64.
"""

import numpy as np

import concourse.bass as bass
import concourse.mybir as mybir
from concourse.tile import TileContext
from concourse.bass_utils import run_bass_kernel_spmd

F32 = mybir.dt.float32
BF16 = mybir.dt.bfloat16


# --------------------------------------------------------------------------
# Compatibility shims for the walrus compiler build in this container:
# 1) EVENT_SEMAPHORE_RANGE_CLEAR (opcode 176) is not understood -> emit
#    per-semaphore EventSemaphore sem-wr-imm 0 instead.
# 2) Instructions with >1 sync waits ("Too many sync wait commands") ->
#    split extra waits onto single-wait EventSemaphore carriers.
# --------------------------------------------------------------------------
def _sem_clear_compat(self, sem):
    nums = (
        list(sem) if isinstance(sem, range)
        else [sem.num if hasattr(sem, "num") else int(sem)]
    )
    inst = None
    for n in nums:
        inst = mybir.InstEventSemaphore(
            name=f"semclr_{self.bass.next_id()}",
            engine=self.engine,
            ins=[],
            outs=[],
            sync_info=mybir.SyncInfo(
                on_wait=[],
                on_update=[
                    mybir.SyncUpdate(
                        sync_type="semaphore",
                        id=n,
                        ant_name=f"semclr{n}",
                        update_mode="sem-wr-imm",
                        update_value=0,
                    )
                ],
            ),
            bass_nofuse=True,
        )
        self.add_instruction(inst)
    return inst


bass.BassGpSimd.sem_clear = _sem_clear_compat


def _split_multi_waits(nc):
    for bb in nc.m.functions[0].blocks:
        new = []
        for inst in bb.instructions:
            si = getattr(inst, "sync_info", None)
            if si is not None and si.on_wait is not None and len(si.on_wait) > 1:
                waits = list(si.on_wait)
                for w in waits[:-1]:
                    carrier = mybir.InstEventSemaphore(
                        name=f"waitsplit_{nc.next_id()}",
                        engine=inst.engine,
                        ins=[],
                        outs=[],
                        sync_info=mybir.SyncInfo(on_wait=[w], on_update=[]),
                        bass_nofuse=True,
                    )
                    new.append(carrier)
                si.on_wait[:] = waits[-1:]
            new.append(inst)
        bb.instructions[:] = new

N_ROWS = 8192
D = 2048
NB = 16          # row blocks
RPB = 512        # rows per block
SLOTS = 10       # blocks cached per core
N_CORES = 8

# Fixed slot-pair list (si = stationary/m-rows, sj = moving/n-cols).
# Ordered so early pairs touch early slots (pipelines with block loads).
PAIRS = [(0, 0), (1, 1), (0, 1)]
for _k in range(2, 9):
    PAIRS += [(0, _k), (1, _k)]
PAIRS += [(1, 9)]
assert len(PAIRS) == 18


def slot_blocks(core):
    """Global block index for each slot on a given core."""
    return [(2 * core + k) % NB for k in range(SLOTS)]


def pair_weight(si, sj):
    """Host-side weight for one slot pair: diag=1, cross d<8 -> 2,
    d=8 cross pairs are computed twice globally -> 1 each."""
    if si == sj:
        return 1.0
    d = sj - si
    return 1.0 if d == 8 else 2.0


def build_program(rpb=RPB, d=D, slots=SLOTS, pairs=PAIRS, psum_bufs=4):
    """Uniform SPMD program. Input: xin [slots, rpb, d] f32 (per-core blocks).
    Output: partials [128, rt*len(pairs)] f32 (per-partition row sums of
    exp(2G-2) per (pair, m-subtile))."""
    rt = rpb // 128   # m-subtiles (psum tiles) per block pair
    kc = d // 128     # contraction chunks
    nc = bass.Bass()
    xin = nc.declare_dram_parameter("xin", [slots, rpb, d], F32, isOutput=False)
    pout = nc.declare_dram_parameter(
        "partials", [128, rt * len(pairs)], F32, isOutput=True
    )

    mult = mybir.AluOpType.mult
    add = mybir.AluOpType.add
    amin = mybir.AluOpType.min
    Exp = mybir.ActivationFunctionType.Exp
    Sqrt = mybir.ActivationFunctionType.Sqrt
    Square = mybir.ActivationFunctionType.Square

    with TileContext(nc) as tc:
        with (
            tc.tile_pool(name="xnt", bufs=1) as xnt_pool,
            tc.tile_pool(name="stage", bufs=4) as stage_pool,
            tc.tile_pool(name="sqs", bufs=2) as sqs_pool,
            tc.tile_pool(name="small", bufs=6) as small_pool,
            tc.tile_pool(name="acc", bufs=1) as acc_pool,
            tc.tile_pool(name="escr", bufs=2) as escr_pool,
            tc.tile_pool(name="gpsum", bufs=psum_bufs, space="PSUM") as gpsum_pool,
            tc.tile_pool(name="epsum", bufs=2, space="PSUM") as epsum_pool,
        ):
            partials = acc_pool.tile([128, rt * len(pairs)], F32, tag="partials")
            bias_t = acc_pool.tile([128, 1], F32, tag="biasneg2")
            nc.vector.memset(bias_t, -2.0)
            xnt = [
                xnt_pool.tile(
                    [128, kc, rpb], BF16, tag=f"xnt{s}", name=f"xnt{s}"
                )
                for s in range(slots)
            ]

            # ---- Phase 0: load + normalize + transpose each slot block ----
            for s in range(slots):
                for r in range(rt):
                    xb = stage_pool.tile([128, d], BF16, tag="xb")
                    # SWDGE casts f32 DRAM -> bf16 SBUF during the DMA.
                    nc.gpsimd.dma_start(
                        out=xb, in_=xin[s, r * 128 : (r + 1) * 128, :]
                    )
                    sqs = sqs_pool.tile([128, d], BF16, tag="sqs")
                    ssq = small_pool.tile([128, 1], F32, tag="ssq")
                    nc.scalar.activation(sqs, xb, Square, accum_out=ssq)
                    nrm = small_pool.tile([128, 1], F32, tag="nrm")
                    nc.scalar.activation(nrm, ssq, Sqrt)
                    rinv = small_pool.tile([128, 1], F32, tag="rinv")
                    nc.vector.reciprocal(rinv, nrm)
                    xn = stage_pool.tile([128, d], BF16, tag="xn")
                    nc.vector.tensor_tensor(
                        out=xn, in0=xb, in1=rinv.to_broadcast((128, d)), op=mult
                    )
                    # xbar transpose: out[p, c, rr] = xn[rr, 128*c + p]
                    nc.sync.dma_start_transpose(
                        out=xnt[s][:, :, r * 128 : (r + 1) * 128], in_=xn
                    )

            # ---- Phase 1: block-pair gram tiles + exp epilogue ----
            for t, (si, sj) in enumerate(pairs):
                for mi in range(rt):
                    g = gpsum_pool.tile([128, rpb], F32, tag="g")
                    for k in range(kc):
                        nc.tensor.matmul(
                            g,
                            xnt[si][:, k, mi * 128 : (mi + 1) * 128],
                            xnt[sj][:, k, :],
                            start=(k == 0),
                            stop=(k == kc - 1),
                        )
                    pcol = t * rt + mi
                    if si == sj:
                        # diagonal elements need the max(d2,0) clamp:
                        # e = min(exp(2G-2), 1)
                        e = epsum_pool.tile([128, rpb], F32, tag="e")
                        nc.scalar.activation(e, g, Exp, bias=bias_t, scale=2.0)
                        scr = escr_pool.tile([128, rpb], F32, tag="scr")
                        nc.vector.tensor_tensor(
                            out=scr, in0=e,
                            in1=nc.const_aps.tensor(1.0, (128, rpb)),
                            op=amin,
                        )
                        nc.vector.tensor_reduce(
                            out=partials[:, pcol : pcol + 1], in_=scr,
                            axis=mybir.AxisListType.X, op=add,
                        )
                    else:
                        edump = epsum_pool.tile([128, rpb], F32, tag="e")
                        nc.scalar.activation(
                            edump, g, Exp, bias=bias_t, scale=2.0,
                            accum_out=partials[:, pcol : pcol + 1],
                        )

            nc.sync.dma_start(out=pout[:, :], in_=partials)
    _split_multi_waits(nc)
    return nc


_PROGRAM_CACHE = {}


def _get_program():
    if "nc" not in _PROGRAM_CACHE:
        _PROGRAM_CACHE["nc"] = build_program()
    return _PROGRAM_CACHE["nc"]


def shard_inputs(x):
    """x: [8192, 2048] f32 -> per-core input dicts."""
    blocks = x.reshape(NB, RPB, D)
    in_maps = []
    for c in range(N_CORES):
        sel = np.ascontiguousarray(blocks[slot_blocks(c)])
        in_maps.append({"xin": sel})
    return in_maps


def reduce_partials(results, rt=RPB // 128):
    """results: list of dicts with 'partials' [128, rt*18] f32 -> scalar."""
    w = np.array([pair_weight(si, sj) for (si, sj) in PAIRS], dtype=np.float64)
    total = 0.0
    for res in results:
        p = res["partials"].astype(np.float64).reshape(128, len(PAIRS), rt)
        # note: partials column layout is t*rt+mi -> reshape (128, 18, rt)
        total += (p.sum(axis=(0, 2)) * w).sum()
    return total / (N_ROWS * (N_ROWS - 1))


def kernel(class_centroid: np.ndarray) -> np.ndarray:
    x = np.asarray(class_centroid, dtype=np.float32)
    assert x.shape == (N_ROWS, D)
    nc = _get_program()
    in_maps = shard_inputs(x)
    out = run_bass_kernel_spmd(nc, in_maps, list(range(N_CORES)))
    total = reduce_partials(out.results)
    return np.float32(total)
